# revision 42
# baseline (speedup 1.0000x reference)
"""DCN cross-network kernel for Trainium2, 8 NeuronCores, pure data parallel.

Math: the reference computes, per layer l (x0, xl: (B, D); w_l, b_l: (D,)):
    s_l = xl @ w_l              # (B,)
    x_{l+1} = x0 * s_l[:, None] + b_l[None, :] + x_l

Writing x_l = x0 * c_l + d_l with per-row scalar c_l and shared vector d_l:
    c_0 = 1, d_0 = 0
    t_l = x0 @ w_l              # per-row, fixed per layer
    u_l = d_l @ w_l             # scalar per layer (host-computed, tiny)
    c_{l+1} = c_l * (1 + t_l) + u_l
    d_{l+1} = d_l + b_l
    out = x0 * c_6 + d_6

So the only large-tensor work is T = x0 @ W^T (one pass over x0) plus a
per-row scale of x0.  On-device per 128-row tile: PE transposes the 8
128x128 blocks (via identity matmul), PE matmuls accumulate
T_tile = x0_tile @ W^T in natural layout, DVE computes
c = prod_l(1 + t_l) (or the Horner recurrence when biases != 0), and DVE
scales x0 by c per partition.  Batch dim is sharded over the 8 cores;
weights are replicated; no collectives.
"""

import os
from contextlib import ExitStack

import numpy as np

import concourse.bass as bass
import concourse.bacc as bacc
import concourse.tile as tile
from concourse import mybir
from concourse.bass_utils import run_bass_kernel_spmd
from concourse.masks import make_identity

P = 128          # partitions
D = 1024         # feature dim
L = 6            # cross layers
KC = D // P      # 8 contraction chunks
N_CORES = 8
F32 = mybir.dt.float32
F32R = mybir.dt.float32r
BF16 = mybir.dt.bfloat16

# Engine split for the PSUM->SBUF pair-copies of transposed blocks (of 4).
SCALAR_COPIES = 4

# Stash of the last BassKernelResults (for test harness introspection).
LAST_RESULTS = None

_BUILD_CACHE = {}


def _build(rows_per_core: int, with_bias: bool, u_vals=None):
    """Build the single-core Bass graph for a (rows_per_core, D) shard."""
    nt = rows_per_core // P
    nc = bacc.Bacc("TRN2", target_bir_lowering=False, debug=False)

    x0_d = nc.dram_tensor("x0", [rows_per_core, D], F32, kind="ExternalInput").ap()
    wt_d = nc.dram_tensor("wt", [P, KC, L], F32, kind="ExternalInput").ap()
    if with_bias:
        d6_d = nc.dram_tensor("d6", [1, D], F32, kind="ExternalInput").ap()
    out_d = nc.dram_tensor("out", [rows_per_core, D], F32, kind="ExternalOutput").ap()

    with tile.TileContext(nc) as tc, ExitStack() as ctx:
        consts = ctx.enter_context(tc.tile_pool(name="consts", bufs=1))
        x0p = ctx.enter_context(tc.tile_pool(name="x0p", bufs=8))
        xbp = ctx.enter_context(tc.tile_pool(name="xbp", bufs=5))
        xtp = ctx.enter_context(tc.tile_pool(name="xtp", bufs=3))
        outp = ctx.enter_context(tc.tile_pool(name="outp", bufs=5))
        small = ctx.enter_context(tc.tile_pool(name="small", bufs=4))
        ps_tr = ctx.enter_context(tc.tile_pool(name="ps_tr", bufs=4, space="PSUM"))
        ps_t = ctx.enter_context(tc.tile_pool(name="ps_t", bufs=2, space="PSUM"))

        ident = consts.tile([P, P], BF16)
        make_identity(nc, ident)
        w_sb = consts.tile([P, KC, L], F32)
        nc.sync.dma_start(out=w_sb, in_=wt_d)
        w_bf = consts.tile([P, KC, L], BF16)
        nc.vector.tensor_copy(out=w_bf, in_=w_sb)
        if with_bias:
            d6_sb = consts.tile([P, D], F32)
            d6_bcast = bass.AP(
                tensor=d6_d.tensor,
                offset=d6_d.offset,
                ap=[[0, P], d6_d.ap[1]],
            )
            nc.sync.dma_start(out=d6_sb, in_=d6_bcast)

        # Super-tiles: partition p holds RPP consecutive rows of the group,
        # so each DMA moves RPP*4KB contiguous per partition.  Each of the
        # RPP row sets gets an independent transpose/dot chain.
        RPP = 2
        x0_v = x0_d.rearrange("(s p j) d -> s p j d", p=P, j=RPP)
        out_v = out_d.rearrange("(s p j) d -> s p j d", p=P, j=RPP)
        nst = nt // RPP
        for t in range(nst):
            x0_t = x0p.tile([P, RPP, D], F32)
            nc.sync.dma_start(out=x0_t, in_=x0_v[t])

            # bf16 copy feeds the PE transpose + dot path; the final
            # per-row scale still reads the f32 original.
            xb = xbp.tile([P, RPP, D], BF16)
            nc.vector.tensor_copy(out=xb, in_=x0_t)

            # Transpose the 128x128 blocks through PE in pairs (one PSUM
            # bank holds 2 blocks), then copy each pair to SBUF at once.
            xt = xtp.tile([P, RPP, KC, P], BF16)
            for j in range(RPP):
                for pr in range(KC // 2):
                    pst = ps_tr.tile([P, 2, P], BF16)
                    for i in range(2):
                        k = pr * 2 + i
                        nc.tensor.transpose(
                            pst[:, i, :], xb[:, j, k * P:(k + 1) * P], ident
                        )
                    if pr < SCALAR_COPIES:
                        nc.scalar.copy(
                            out=xt[:, j, pr * 2:(pr + 1) * 2, :], in_=pst
                        )
                    else:
                        nc.vector.tensor_copy(
                            out=xt[:, j, pr * 2:(pr + 1) * 2, :], in_=pst
                        )

            o_t = outp.tile([P, RPP, D], F32)
            for j in range(RPP):
                # T = x0 @ W^T for this row set, natural layout.
                tp = ps_t.tile([P, L], F32)
                for k in range(KC):
                    nc.tensor.matmul(
                        tp,
                        lhsT=xt[:, j, k, :],
                        rhs=w_bf[:, k, :],
                        start=(k == 0),
                        stop=(k == KC - 1),
                    )

                # f = 1 + t (also moves PSUM -> SBUF)
                f_sb = small.tile([P, L], F32)
                nc.vector.tensor_scalar_add(f_sb, tp, 1.0)

                c = small.tile([P, 1], F32)
                if not with_bias:
                    # c = prod_l (1 + t_l)
                    nc.vector.tensor_reduce(
                        c, f_sb, axis=mybir.AxisListType.X,
                        op=mybir.AluOpType.mult,
                    )
                else:
                    # Horner: c <- c * f_l + u_l
                    nc.vector.memset(c, 1.0)
                    for l in range(L):
                        nc.vector.tensor_scalar(
                            out=c,
                            in0=c,
                            scalar1=f_sb[:, l:l + 1],
                            scalar2=float(u_vals[l]),
                            op0=mybir.AluOpType.mult,
                            op1=mybir.AluOpType.add,
                        )

                nc.vector.tensor_scalar_mul(o_t[:, j, :], x0_t[:, j, :], c)
                if with_bias:
                    nc.vector.tensor_add(o_t[:, j, :], o_t[:, j, :], d6_sb)
            # out-DMAs issue from gpsimd so their waits never block the
            # sync engine's input stream.
            nc.gpsimd.dma_start(out=out_v[t], in_=o_t)

    nc.compile()
    return nc


def kernel(x0: np.ndarray, weights: np.ndarray, biases: np.ndarray) -> np.ndarray:
    global LAST_RESULTS
    x0 = np.ascontiguousarray(x0, dtype=np.float32)
    weights = np.ascontiguousarray(weights, dtype=np.float32)
    biases = np.ascontiguousarray(biases, dtype=np.float32)

    B = x0.shape[0]
    rows_per_core = B // N_CORES
    with_bias = bool(np.any(biases))

    # wt[p, k, l] = weights[l, 128k + p]
    wt = np.ascontiguousarray(weights.T.reshape(KC, P, L).transpose(1, 0, 2))

    u_vals = None
    d6 = None
    if with_bias:
        d = np.zeros(D, np.float64)
        u_vals = []
        for l in range(L):
            u_vals.append(float(d @ weights[l].astype(np.float64)))
            d = d + biases[l]
        d6 = d.astype(np.float32).reshape(1, D)

    key = (rows_per_core, with_bias, None if u_vals is None else tuple(u_vals))
    if key not in _BUILD_CACHE:
        _BUILD_CACHE[key] = _build(rows_per_core, with_bias, u_vals)
    nc = _BUILD_CACHE[key]

    in_maps = []
    for i in range(N_CORES):
        m = {"x0": x0[i * rows_per_core:(i + 1) * rows_per_core], "wt": wt}
        if with_bias:
            m["d6"] = d6
        in_maps.append(m)

    trace = bool(os.environ.get("KERNEL_TRACE"))
    try:
        res = run_bass_kernel_spmd(
            nc, in_maps, core_ids=list(range(N_CORES)), trace=trace
        )
    except Exception:
        if not trace:
            raise
        res = run_bass_kernel_spmd(nc, in_maps, core_ids=list(range(N_CORES)))
    LAST_RESULTS = res
    out = np.concatenate([res.results[i]["out"] for i in range(N_CORES)], axis=0)
    return out.astype(np.float32)


# revision 43
# speedup vs baseline: 1.0036x; 1.0036x over previous
"""DCN cross-network kernel for Trainium2, 8 NeuronCores, pure data parallel.

Math: the reference computes, per layer l (x0, xl: (B, D); w_l, b_l: (D,)):
    s_l = xl @ w_l              # (B,)
    x_{l+1} = x0 * s_l[:, None] + b_l[None, :] + x_l

Writing x_l = x0 * c_l + d_l with per-row scalar c_l and shared vector d_l:
    c_0 = 1, d_0 = 0
    t_l = x0 @ w_l              # per-row, fixed per layer
    u_l = d_l @ w_l             # scalar per layer (host-computed, tiny)
    c_{l+1} = c_l * (1 + t_l) + u_l
    d_{l+1} = d_l + b_l
    out = x0 * c_6 + d_6

So the only large-tensor work is T = x0 @ W^T (one pass over x0) plus a
per-row scale of x0.  On-device per 128-row tile: PE transposes the 8
128x128 blocks (via identity matmul), PE matmuls accumulate
T_tile = x0_tile @ W^T in natural layout, DVE computes
c = prod_l(1 + t_l) (or the Horner recurrence when biases != 0), and DVE
scales x0 by c per partition.  Batch dim is sharded over the 8 cores;
weights are replicated; no collectives.
"""

import os
from contextlib import ExitStack

import numpy as np

import concourse.bass as bass
import concourse.bacc as bacc
import concourse.tile as tile
from concourse import mybir
from concourse.bass_utils import run_bass_kernel_spmd
from concourse.masks import make_identity

P = 128          # partitions
D = 1024         # feature dim
L = 6            # cross layers
KC = D // P      # 8 contraction chunks
N_CORES = 8
F32 = mybir.dt.float32
F32R = mybir.dt.float32r
BF16 = mybir.dt.bfloat16

# Engine split for the PSUM->SBUF pair-copies of transposed blocks (of 4).
SCALAR_COPIES = 3

# Stash of the last BassKernelResults (for test harness introspection).
LAST_RESULTS = None

_BUILD_CACHE = {}


def _build(rows_per_core: int, with_bias: bool, u_vals=None):
    """Build the single-core Bass graph for a (rows_per_core, D) shard."""
    nt = rows_per_core // P
    nc = bacc.Bacc("TRN2", target_bir_lowering=False, debug=False)

    x0_d = nc.dram_tensor("x0", [rows_per_core, D], F32, kind="ExternalInput").ap()
    wt_d = nc.dram_tensor("wt", [P, KC, L], F32, kind="ExternalInput").ap()
    if with_bias:
        d6_d = nc.dram_tensor("d6", [1, D], F32, kind="ExternalInput").ap()
    out_d = nc.dram_tensor("out", [rows_per_core, D], F32, kind="ExternalOutput").ap()

    with tile.TileContext(nc) as tc, ExitStack() as ctx:
        consts = ctx.enter_context(tc.tile_pool(name="consts", bufs=1))
        x0p = ctx.enter_context(tc.tile_pool(name="x0p", bufs=6))
        xbp = ctx.enter_context(tc.tile_pool(name="xbp", bufs=4))
        xtp = ctx.enter_context(tc.tile_pool(name="xtp", bufs=3))
        outp = ctx.enter_context(tc.tile_pool(name="outp", bufs=4))
        small = ctx.enter_context(tc.tile_pool(name="small", bufs=4))
        ps_tr = ctx.enter_context(tc.tile_pool(name="ps_tr", bufs=4, space="PSUM"))
        ps_t = ctx.enter_context(tc.tile_pool(name="ps_t", bufs=2, space="PSUM"))

        ident = consts.tile([P, P], BF16)
        make_identity(nc, ident)
        w_sb = consts.tile([P, KC, L], F32)
        nc.sync.dma_start(out=w_sb, in_=wt_d)
        w_bf = consts.tile([P, KC, L], BF16)
        nc.vector.tensor_copy(out=w_bf, in_=w_sb)
        if with_bias:
            d6_sb = consts.tile([P, D], F32)
            d6_bcast = bass.AP(
                tensor=d6_d.tensor,
                offset=d6_d.offset,
                ap=[[0, P], d6_d.ap[1]],
            )
            nc.sync.dma_start(out=d6_sb, in_=d6_bcast)

        # Super-tiles: partition p holds RPP consecutive rows of the group,
        # so each DMA moves RPP*4KB contiguous per partition.  Each of the
        # RPP row sets gets an independent transpose/dot chain.
        RPP = 2
        x0_v = x0_d.rearrange("(s p j) d -> s p j d", p=P, j=RPP)
        out_v = out_d.rearrange("(s p j) d -> s p j d", p=P, j=RPP)
        nst = nt // RPP
        for t in range(nst):
            x0_t = x0p.tile([P, RPP, D], F32)
            nc.sync.dma_start(out=x0_t, in_=x0_v[t])

            # bf16 copy feeds the PE transpose + dot path; the final
            # per-row scale still reads the f32 original.
            xb = xbp.tile([P, RPP, D], BF16)
            nc.vector.tensor_copy(out=xb, in_=x0_t)

            # Transpose the 128x128 blocks through PE in pairs (one PSUM
            # bank holds 2 blocks), then copy each pair to SBUF at once.
            xt = xtp.tile([P, RPP, KC, P], BF16)
            for j in range(RPP):
                for pr in range(KC // 2):
                    pst = ps_tr.tile([P, 2, P], BF16)
                    for i in range(2):
                        k = pr * 2 + i
                        nc.tensor.transpose(
                            pst[:, i, :], xb[:, j, k * P:(k + 1) * P], ident
                        )
                    if pr < SCALAR_COPIES:
                        nc.scalar.copy(
                            out=xt[:, j, pr * 2:(pr + 1) * 2, :], in_=pst
                        )
                    else:
                        nc.vector.tensor_copy(
                            out=xt[:, j, pr * 2:(pr + 1) * 2, :], in_=pst
                        )

            o_t = outp.tile([P, RPP, D], F32)
            for j in range(RPP):
                # T = x0 @ W^T for this row set, natural layout.
                tp = ps_t.tile([P, L], F32)
                for k in range(KC):
                    nc.tensor.matmul(
                        tp,
                        lhsT=xt[:, j, k, :],
                        rhs=w_bf[:, k, :],
                        start=(k == 0),
                        stop=(k == KC - 1),
                    )

                # f = 1 + t (also moves PSUM -> SBUF)
                f_sb = small.tile([P, L], F32)
                nc.vector.tensor_scalar_add(f_sb, tp, 1.0)

                c = small.tile([P, 1], F32)
                if not with_bias:
                    # c = prod_l (1 + t_l)
                    nc.vector.tensor_reduce(
                        c, f_sb, axis=mybir.AxisListType.X,
                        op=mybir.AluOpType.mult,
                    )
                else:
                    # Horner: c <- c * f_l + u_l
                    nc.vector.memset(c, 1.0)
                    for l in range(L):
                        nc.vector.tensor_scalar(
                            out=c,
                            in0=c,
                            scalar1=f_sb[:, l:l + 1],
                            scalar2=float(u_vals[l]),
                            op0=mybir.AluOpType.mult,
                            op1=mybir.AluOpType.add,
                        )

                nc.vector.tensor_scalar_mul(o_t[:, j, :], x0_t[:, j, :], c)
                if with_bias:
                    nc.vector.tensor_add(o_t[:, j, :], o_t[:, j, :], d6_sb)
            # out-DMAs issue from gpsimd so their waits never block the
            # sync engine's input stream.
            nc.gpsimd.dma_start(out=out_v[t], in_=o_t)

    nc.compile()
    return nc


def kernel(x0: np.ndarray, weights: np.ndarray, biases: np.ndarray) -> np.ndarray:
    global LAST_RESULTS
    x0 = np.ascontiguousarray(x0, dtype=np.float32)
    weights = np.ascontiguousarray(weights, dtype=np.float32)
    biases = np.ascontiguousarray(biases, dtype=np.float32)

    B = x0.shape[0]
    rows_per_core = B // N_CORES
    with_bias = bool(np.any(biases))

    # wt[p, k, l] = weights[l, 128k + p]
    wt = np.ascontiguousarray(weights.T.reshape(KC, P, L).transpose(1, 0, 2))

    u_vals = None
    d6 = None
    if with_bias:
        d = np.zeros(D, np.float64)
        u_vals = []
        for l in range(L):
            u_vals.append(float(d @ weights[l].astype(np.float64)))
            d = d + biases[l]
        d6 = d.astype(np.float32).reshape(1, D)

    key = (rows_per_core, with_bias, None if u_vals is None else tuple(u_vals))
    if key not in _BUILD_CACHE:
        _BUILD_CACHE[key] = _build(rows_per_core, with_bias, u_vals)
    nc = _BUILD_CACHE[key]

    in_maps = []
    for i in range(N_CORES):
        m = {"x0": x0[i * rows_per_core:(i + 1) * rows_per_core], "wt": wt}
        if with_bias:
            m["d6"] = d6
        in_maps.append(m)

    trace = bool(os.environ.get("KERNEL_TRACE"))
    try:
        res = run_bass_kernel_spmd(
            nc, in_maps, core_ids=list(range(N_CORES)), trace=trace
        )
    except Exception:
        if not trace:
            raise
        res = run_bass_kernel_spmd(nc, in_maps, core_ids=list(range(N_CORES)))
    LAST_RESULTS = res
    out = np.concatenate([res.results[i]["out"] for i in range(N_CORES)], axis=0)
    return out.astype(np.float32)


# revision 44
# speedup vs baseline: 1.0083x; 1.0046x over previous
"""DCN cross-network kernel for Trainium2, 8 NeuronCores, pure data parallel.

Math: the reference computes, per layer l (x0, xl: (B, D); w_l, b_l: (D,)):
    s_l = xl @ w_l              # (B,)
    x_{l+1} = x0 * s_l[:, None] + b_l[None, :] + x_l

Writing x_l = x0 * c_l + d_l with per-row scalar c_l and shared vector d_l:
    c_0 = 1, d_0 = 0
    t_l = x0 @ w_l              # per-row, fixed per layer
    u_l = d_l @ w_l             # scalar per layer (host-computed, tiny)
    c_{l+1} = c_l * (1 + t_l) + u_l
    d_{l+1} = d_l + b_l
    out = x0 * c_6 + d_6

So the only large-tensor work is T = x0 @ W^T (one pass over x0) plus a
per-row scale of x0.  On-device per 128-row tile: PE transposes the 8
128x128 blocks (via identity matmul), PE matmuls accumulate
T_tile = x0_tile @ W^T in natural layout, DVE computes
c = prod_l(1 + t_l) (or the Horner recurrence when biases != 0), and DVE
scales x0 by c per partition.  Batch dim is sharded over the 8 cores;
weights are replicated; no collectives.
"""

import os
from contextlib import ExitStack

import numpy as np

import concourse.bass as bass
import concourse.bacc as bacc
import concourse.tile as tile
from concourse import mybir
from concourse.bass_utils import run_bass_kernel_spmd
from concourse.masks import make_identity

P = 128          # partitions
D = 1024         # feature dim
L = 6            # cross layers
KC = D // P      # 8 contraction chunks
N_CORES = 8
F32 = mybir.dt.float32
F32R = mybir.dt.float32r
BF16 = mybir.dt.bfloat16
F16 = mybir.dt.float16

# Engine split for the PSUM->SBUF pair-copies of transposed blocks (of 4).
SCALAR_COPIES = 3

# Stash of the last BassKernelResults (for test harness introspection).
LAST_RESULTS = None

_BUILD_CACHE = {}


def _build(rows_per_core: int, with_bias: bool, u_vals=None, half=None):
    """Build the single-core Bass graph for a (rows_per_core, D) shard."""
    nt = rows_per_core // P
    if half is None:
        half = F16
    nc = bacc.Bacc("TRN2", target_bir_lowering=False, debug=False)

    x0_d = nc.dram_tensor("x0", [rows_per_core, D], F32, kind="ExternalInput").ap()
    wt_d = nc.dram_tensor("wt", [P, KC, L], F32, kind="ExternalInput").ap()
    if with_bias:
        d6_d = nc.dram_tensor("d6", [1, D], F32, kind="ExternalInput").ap()
    out_d = nc.dram_tensor("out", [rows_per_core, D], F32, kind="ExternalOutput").ap()

    with tile.TileContext(nc) as tc, ExitStack() as ctx:
        consts = ctx.enter_context(tc.tile_pool(name="consts", bufs=1))
        x0p = ctx.enter_context(tc.tile_pool(name="x0p", bufs=6))
        xbp = ctx.enter_context(tc.tile_pool(name="xbp", bufs=4))
        xtp = ctx.enter_context(tc.tile_pool(name="xtp", bufs=3))
        outp = ctx.enter_context(tc.tile_pool(name="outp", bufs=4))
        small = ctx.enter_context(tc.tile_pool(name="small", bufs=4))
        ps_tr = ctx.enter_context(tc.tile_pool(name="ps_tr", bufs=4, space="PSUM"))
        ps_t = ctx.enter_context(tc.tile_pool(name="ps_t", bufs=2, space="PSUM"))

        ident = consts.tile([P, P], half)
        make_identity(nc, ident)
        w_sb = consts.tile([P, KC, L], F32)
        nc.sync.dma_start(out=w_sb, in_=wt_d)
        w_bf = consts.tile([P, KC, L], half)
        nc.vector.tensor_copy(out=w_bf, in_=w_sb)
        if with_bias:
            d6_sb = consts.tile([P, D], F32)
            d6_bcast = bass.AP(
                tensor=d6_d.tensor,
                offset=d6_d.offset,
                ap=[[0, P], d6_d.ap[1]],
            )
            nc.sync.dma_start(out=d6_sb, in_=d6_bcast)

        # Super-tiles: partition p holds RPP consecutive rows of the group,
        # so each DMA moves RPP*4KB contiguous per partition.  Each of the
        # RPP row sets gets an independent transpose/dot chain.
        RPP = 2
        x0_v = x0_d.rearrange("(s p j) d -> s p j d", p=P, j=RPP)
        out_v = out_d.rearrange("(s p j) d -> s p j d", p=P, j=RPP)
        nst = nt // RPP
        for t in range(nst):
            x0_t = x0p.tile([P, RPP, D], F32)
            nc.sync.dma_start(out=x0_t, in_=x0_v[t])

            # bf16 copy feeds the PE transpose + dot path; the final
            # per-row scale still reads the f32 original.
            xb = xbp.tile([P, RPP, D], half)
            nc.vector.tensor_copy(out=xb, in_=x0_t)

            # Transpose the 128x128 blocks through PE in pairs (one PSUM
            # bank holds 2 blocks), then copy each pair to SBUF at once.
            xt = xtp.tile([P, RPP, KC, P], half)
            for j in range(RPP):
                for pr in range(KC // 2):
                    pst = ps_tr.tile([P, 2, P], half)
                    for i in range(2):
                        k = pr * 2 + i
                        nc.tensor.transpose(
                            pst[:, i, :], xb[:, j, k * P:(k + 1) * P], ident
                        )
                    if pr < SCALAR_COPIES:
                        nc.scalar.copy(
                            out=xt[:, j, pr * 2:(pr + 1) * 2, :], in_=pst
                        )
                    else:
                        nc.vector.tensor_copy(
                            out=xt[:, j, pr * 2:(pr + 1) * 2, :], in_=pst
                        )

            o_t = outp.tile([P, RPP, D], F32)
            for j in range(RPP):
                # T = x0 @ W^T for this row set, natural layout.
                tp = ps_t.tile([P, L], F32)
                for k in range(KC):
                    nc.tensor.matmul(
                        tp,
                        lhsT=xt[:, j, k, :],
                        rhs=w_bf[:, k, :],
                        start=(k == 0),
                        stop=(k == KC - 1),
                    )

                # f = 1 + t (also moves PSUM -> SBUF)
                f_sb = small.tile([P, L], F32)
                nc.vector.tensor_scalar_add(f_sb, tp, 1.0)

                c = small.tile([P, 1], F32)
                if not with_bias:
                    # c = prod_l (1 + t_l)
                    nc.vector.tensor_reduce(
                        c, f_sb, axis=mybir.AxisListType.X,
                        op=mybir.AluOpType.mult,
                    )
                else:
                    # Horner: c <- c * f_l + u_l
                    nc.vector.memset(c, 1.0)
                    for l in range(L):
                        nc.vector.tensor_scalar(
                            out=c,
                            in0=c,
                            scalar1=f_sb[:, l:l + 1],
                            scalar2=float(u_vals[l]),
                            op0=mybir.AluOpType.mult,
                            op1=mybir.AluOpType.add,
                        )

                nc.vector.tensor_scalar_mul(o_t[:, j, :], x0_t[:, j, :], c)
                if with_bias:
                    nc.vector.tensor_add(o_t[:, j, :], o_t[:, j, :], d6_sb)
            # out-DMAs issue from gpsimd so their waits never block the
            # sync engine's input stream.
            nc.gpsimd.dma_start(out=out_v[t], in_=o_t)

    nc.compile()
    return nc


def kernel(x0: np.ndarray, weights: np.ndarray, biases: np.ndarray) -> np.ndarray:
    global LAST_RESULTS
    x0 = np.ascontiguousarray(x0, dtype=np.float32)
    weights = np.ascontiguousarray(weights, dtype=np.float32)
    biases = np.ascontiguousarray(biases, dtype=np.float32)

    B = x0.shape[0]
    rows_per_core = B // N_CORES
    with_bias = bool(np.any(biases))

    # wt[p, k, l] = weights[l, 128k + p]
    wt = np.ascontiguousarray(weights.T.reshape(KC, P, L).transpose(1, 0, 2))

    u_vals = None
    d6 = None
    if with_bias:
        d = np.zeros(D, np.float64)
        u_vals = []
        for l in range(L):
            u_vals.append(float(d @ weights[l].astype(np.float64)))
            d = d + biases[l]
        d6 = d.astype(np.float32).reshape(1, D)

    # f16 has a 10-bit mantissa (4x tighter dots than bf16) and is safe
    # unless values approach the f16 range limit.
    half = F16 if float(np.max(np.abs(x0))) < 1e3 else BF16
    key = (rows_per_core, with_bias,
           None if u_vals is None else tuple(u_vals), str(half))
    if key not in _BUILD_CACHE:
        _BUILD_CACHE[key] = _build(rows_per_core, with_bias, u_vals, half)
    nc = _BUILD_CACHE[key]

    in_maps = []
    for i in range(N_CORES):
        m = {"x0": x0[i * rows_per_core:(i + 1) * rows_per_core], "wt": wt}
        if with_bias:
            m["d6"] = d6
        in_maps.append(m)

    trace = bool(os.environ.get("KERNEL_TRACE"))
    try:
        res = run_bass_kernel_spmd(
            nc, in_maps, core_ids=list(range(N_CORES)), trace=trace
        )
    except Exception:
        if not trace:
            raise
        res = run_bass_kernel_spmd(nc, in_maps, core_ids=list(range(N_CORES)))
    LAST_RESULTS = res
    out = np.concatenate([res.results[i]["out"] for i in range(N_CORES)], axis=0)
    return out.astype(np.float32)
